# revision 17
# baseline (speedup 1.0000x reference)
"""Trainium2 Bass kernel for NeuralECMModel (gnn_message_passing).

Math (per node n):
  ent  = entity_emb @ Wp.T + bp                                   [N,50]
  node = einsum('ni,oij,nj->no', q, Wbil, ent) + bbil             [N,50]
  wtext= sum_k s[n,k]*nbr[n,k,:] + s[n,63]*node[n,:]              [N,50]
  agg  = wtext @ Wg.T                                             [N,50]
  out  = elu(agg + g_bias) @ Wr.T + br                            [N,1]

Key restructuring (vs naive): Wg is folded into both branches so `agg`
is accumulated directly in PSUM by the PE:
  agg[p,n] = sum_{(d,k)} Wg[p,d]*s[n,k]*nbr[n,k,d]        (PE contraction
             over 25 chunks of the transposed neighbor stream)
           + s63[n]*(q Wtil[p] ent + bbilg[p])            (row-major bilinear,
             transpose-matmul-accumulated into the same PSUM tile)
  with Wtil[p,i,j] = sum_o Wg[p,o]*Wbil[o,i,j], bbilg = Wg @ bbil.

This removes the k-tree reduction from the vector engines entirely; the
score multiply is ONE full-rate bf16 DVE op per 500-node macro tile.

Sharding: pure data parallel over nodes, N=20000 -> 2500 nodes/core x 8.
"""

import numpy as np
import ml_dtypes

import concourse.bass as bass
import concourse.bacc as bacc
import concourse.tile as tile
import concourse.mybir as mybir
from concourse.bass_utils import run_bass_kernel_spmd
from concourse.masks import make_identity

F32 = mybir.dt.float32
BF16 = mybir.dt.bfloat16
OP = mybir.AluOpType
AF = mybir.ActivationFunctionType
AX = mybir.AxisListType

N_CORES = 8
N = 20000
NLOC = N // N_CORES   # 2500
K = 63
D = 50
E = 300
EA = 304              # padded augmented entity rows (300 + ones + 3 zero)
P = 128
SR = 125              # bilinear sub-tile rows
NCH = 25              # neighbor (d,k) chunks of 126 rows
CR = 2 * K            # 126 rows per chunk (2 d's x 63 k's)
OJ = D * D            # 2500

RW = 500              # agg region width (one PSUM bank: 500 f32)
# per-sub-tile U o-chunk assignment (n_direct_psum_dve, n_sbuf_dve, n_pool)
UPLAN = ((0, 5, 0),)   # all U chunks: ACT egress copy -> DVE mul+reduce
# snbr chunk engine: True -> Pool, False -> DVE
SNBR_POOL = tuple(False for _ in range(25))
# U psum chunk width in o's (each *D wide); one PSUM bank per chunk so every
# matmul output is bank-aligned
UW = 10

_CACHE = {}


def _tree_levels(w):
    """Pairwise-halving splits: [(hi, lo), ...] meaning x[0:hi] += x[lo:lo+hi]."""
    out = []
    while w > 1:
        lo = (w + 1) // 2
        hi = w - lo
        out.append((hi, lo))
        w = lo
    return out


def build_program(br_val: float, skip_bil=False, skip_nbr=False, dump_ng=False,
                  repeat=1, uplan=None, dve_tree=False, snbr_pool=None):
    """repeat>1 wraps the whole per-run body (including all input streaming)
    in a hardware loop - used by test.py to time steady-state per-execution
    HW time with the ~3.6ms axon launch round-trip amortized away.

    v2 structure (single pass over all NLOC nodes):
      - neighbor stream DMA'd chunk-major: 25 chunks of [126, 2500] with 5KB
        contiguous rows (vs 0.5-1KB rows of the macro-tile slicing).
      - agg accumulated in 5 persistent PSUM bank regions [50, 500] covering
        all 2500 nodes; ONE tail (elu + Wr head) at the end.
      - bilinear j-reduction on DVE via a single tensor_reduce(X) instead of
        a 6-instruction pairwise tree; Pool keeps the tree for its o-share.
      - per-sub-tile engine assignment of the 5 U o-chunks: chunk 0 is
        consumed by DVE straight from PSUM (saves the ACT egress copy),
        the rest split DVE/Pool via an alternating pattern.
    """
    nc = bacc.Bacc("TRN2", debug=False, num_devices=N_CORES)

    # ---- per-core inputs (layouts unchanged from v1) ----
    t_nbrT = nc.dram_tensor("nbrT", [NCH * CR, NLOC], BF16, kind="ExternalInput")
    t_f32p = nc.dram_tensor("f32p", [SR, NLOC // SR + 1], F32, kind="ExternalInput")
    t_qW = nc.dram_tensor("qW", [D, NLOC + OJ], BF16, kind="ExternalInput")
    t_entT0 = nc.dram_tensor("entT0", [128, NLOC], BF16, kind="ExternalInput")
    t_entT1 = nc.dram_tensor("entT1", [128, NLOC], BF16, kind="ExternalInput")
    t_entT2 = nc.dram_tensor("entT2", [EA - 256, NLOC], BF16, kind="ExternalInput")
    t_wpack = nc.dram_tensor("wpack", [128, 100 + D + 1 + D], BF16, kind="ExternalInput")
    t_sWgK = nc.dram_tensor("sWgK", [CR, NLOC + NCH * D], BF16, kind="ExternalInput")
    t_s63r = nc.dram_tensor("s63r", [1, NLOC], BF16, kind="ExternalInput")
    t_out = nc.dram_tensor("out", [1, NLOC], F32, kind="ExternalOutput")

    NREG = NLOC // RW          # 5 agg regions of 500 nodes (1 PSUM bank each)
    NSUB = NLOC // SR          # 20 bilinear sub-tiles
    SPR = RW // SR             # 4 sub-tiles per region

    with tile.TileContext(nc) as tc:
        with (
            tc.tile_pool(name="res", bufs=1) as res,
            tc.tile_pool(name="nbrp", bufs=6) as nbrp,
            tc.tile_pool(name="snbrp", bufs=4) as snbrp,
            tc.tile_pool(name="usbd", bufs=3) as usbd_p,
            tc.tile_pool(name="usbg", bufs=3) as usbg_p,
            tc.tile_pool(name="prodd", bufs=3) as prodd_p,
            tc.tile_pool(name="prodg", bufs=3) as prodg_p,
            tc.tile_pool(name="small", bufs=8) as small,
            tc.tile_pool(name="nodegs", bufs=20) as nodegs,
            tc.tile_pool(name="tailp", bufs=3) as tailp,
            tc.tile_pool(name="ps_agg", bufs=1, space="PSUM") as ps_agg,
            tc.tile_pool(name="ps_u", bufs=2, space="PSUM") as ps_u,
            tc.tile_pool(name="ps_misc", bufs=1, space="PSUM") as ps_misc,
            nc.allow_low_precision(reason="bf16 bilinear reduce; node term is ~10% of signal"),
        ):
            ident_sb = res.tile([P, P], BF16)
            make_identity(nc, ident_sb)

            def emit_body():
                # ---- residents: sWgK first (feeds snbr_0, DVE's first op) ----
                sWgK_sb = res.tile([CR, NLOC + NCH * D], BF16)
                nc.sync.dma_start(out=sWgK_sb, in_=t_sWgK[:])
                qW_sb = res.tile([D, NLOC + OJ], BF16)
                nc.sync.dma_start(out=qW_sb, in_=t_qW[:])
                wpack_sb = res.tile([P, 100 + D + 1 + D], BF16)
                nc.sync.dma_start(out=wpack_sb, in_=t_wpack[:])
                f32p_sb = res.tile([SR, NLOC // SR + 1], F32)
                nc.sync.dma_start(out=f32p_sb, in_=t_f32p[:])
                entT_sb = [
                    res.tile([128, NLOC], BF16, tag="entT0", name="entT0_sb"),
                    res.tile([128, NLOC], BF16, tag="entT1", name="entT1_sb"),
                    res.tile([EA - 256, NLOC], BF16, tag="entT2", name="entT2_sb"),
                ]
                for sb, t in zip(entT_sb, (t_entT0, t_entT1, t_entT2)):
                    nc.sync.dma_start(out=sb, in_=t[:])
                s63r_sb = res.tile([1, NLOC], BF16)
                nc.sync.dma_start(out=s63r_sb, in_=t_s63r[:])
                out_row = res.tile([1, NLOC], F32)

                sTrep_sb = sWgK_sb[:, 0:NLOC]
                WgK_sb = sWgK_sb[:, NLOC : NLOC + NCH * D]
                qT_sb = qW_sb[:, 0:NLOC]
                W2til_sb = qW_sb[:, NLOC : NLOC + OJ]
                WpT_sb = [
                    wpack_sb[:, 0:D],
                    wpack_sb[:, D : 2 * D],
                    wpack_sb[0 : EA - 256, 151 : 151 + D],
                ]
                bbilg_rep = wpack_sb[:, 100:150]
                WrT_col = wpack_sb[0:D, 150:151]
                s63p_sb = f32p_sb[:, 0 : NLOC // SR]
                gb_sb = f32p_sb[0:D, NLOC // SR : NLOC // SR + 1]

                # ---- neighbor chunk DMAs (self-throttled by pool bufs) ----
                nbr_c = []
                for c in range(NCH if not skip_nbr else 0):
                    t = nbrp.tile([CR, NLOC], BF16, tag="nbr", name=f"nbr{c}")
                    nc.sync.dma_start(out=t, in_=t_nbrT[c * CR : (c + 1) * CR, :])
                    nbr_c.append(t)

                # persistent agg regions
                aggR = [
                    ps_agg.tile([D, RW], F32, tag=f"agg{r}", name=f"agg{r}")
                    for r in range(NREG)
                ]

                def emit_snbr(c):
                    st_ = snbrp.tile([CR, NLOC], BF16, tag="snbr", name=f"snbr{c}")
                    sp = snbr_pool if snbr_pool is not None else SNBR_POOL
                    eng = nc.gpsimd if sp[c] else nc.vector
                    eng.tensor_mul(st_, nbr_c[c], sTrep_sb)
                    return st_

                def emit_nbr_mm(c, snbr_t):
                    for r in range(NREG):
                        nc.tensor.matmul(
                            aggR[r],
                            WgK_sb[:, c * D : (c + 1) * D],
                            snbr_t[:, r * RW : (r + 1) * RW],
                            start=(c == 0),
                            stop=False,
                        )

                # ---- interleaved bilinear sub-tiles + neighbor chunks ----
                ci = 0  # next neighbor chunk to emit
                node_gbs = []

                def emit_chunk():
                    nonlocal ci
                    if not skip_nbr and ci < NCH:
                        emit_nbr_mm(ci, emit_snbr(ci))
                        ci += 1

                for st in range(NSUB if not skip_bil else 0):
                    r0 = st * SR
                    rs = slice(r0, r0 + SR)

                    ent_ps = ps_misc.tile([SR, D], F32, tag="misc", name="ent_ps")
                    for c in range(3):
                        nc.tensor.matmul(
                            ent_ps,
                            entT_sb[c][:, rs],
                            WpT_sb[c],
                            start=(c == 0),
                            stop=(c == 2),
                        )
                    ents = small.tile([SR, D], BF16, tag="ents")
                    nc.scalar.activation(
                        out=ents, in_=ent_ps, func=AF.Copy,
                        scale=s63p_sb[:, st : st + 1],
                    )
                    ev = ents.unsqueeze(1).broadcast_to([SR, D, D])

                    # U chunk plan for this sub-tile: (n_direct, n_dve, n_pool)
                    _up = uplan if uplan is not None else UPLAN
                    nD, nV, nG = _up[st % len(_up)]
                    node_g = nodegs.tile([SR, D], BF16, tag="node_g",
                                         name=f"node_g{st}")
                    node_gbs.append(node_g)

                    # direct-PSUM chunks: DVE consumes u_ps f32 (no ACT copy)
                    for k in range(nD):
                        o0 = k * UW
                        u_ps = ps_u.tile([SR, UW * D], F32, tag="ups")
                        nc.tensor.matmul(
                            u_ps, qT_sb[:, rs],
                            W2til_sb[:, o0 * D : (o0 + UW) * D],
                            start=True, stop=True,
                        )
                        prodd = prodd_p.tile([SR, UW * D], BF16, tag="prodD")
                        pv = prodd.rearrange("p (o j) -> p o j", o=UW)
                        nc.vector.tensor_mul(
                            pv, u_ps.rearrange("p (o j) -> p o j", o=UW),
                            ev[:, o0 : o0 + UW],
                        )
                        nc.vector.tensor_reduce(
                            node_g[:, o0 : o0 + UW].unsqueeze(2), pv,
                            axis=AX.X, op=OP.add,
                        )

                    # DVE-via-SBUF chunks (ACT egress copy, batched mul+reduce)
                    oV = nD * UW
                    wV = nV * UW
                    if nV:
                        usbd = usbd_p.tile([SR, wV * D], BF16, tag="usbd")
                        for k in range(nV):
                            o0 = (nD + k) * UW
                            u_ps = ps_u.tile([SR, UW * D], F32, tag="ups")
                            nc.tensor.matmul(
                                u_ps, qT_sb[:, rs],
                                W2til_sb[:, o0 * D : (o0 + UW) * D],
                                start=True, stop=True,
                            )
                            nc.scalar.copy(
                                out=usbd[:, k * UW * D : (k + 1) * UW * D],
                                in_=u_ps,
                            )
                        prodv = prodd_p.tile([SR, wV * D], BF16, tag="prodV")
                        # j-major layout: prod[p, j*D+o]; tree adds are flat 2D
                        pv = prodv.rearrange("p (j o) -> p j o", o=D)
                        evj = ents.unsqueeze(2).broadcast_to([SR, D, D])
                        nc.vector.tensor_mul(
                            pv, usbd.rearrange("p (j o) -> p j o", o=D), evj,
                        )
                        w = D
                        for hi, lo in _tree_levels(D):
                            if w == 2:
                                nc.vector.tensor_add(
                                    node_g, prodv[:, 0:D], prodv[:, D : 2 * D],
                                )
                            else:
                                nc.vector.tensor_add(
                                    prodv[:, 0 : hi * D],
                                    prodv[:, 0 : hi * D],
                                    prodv[:, lo * D : (lo + hi) * D],
                                )
                            w = lo

                    # Pool chunks (ACT egress copy, mul + pairwise tree)
                    oG = (nD + nV) * UW
                    wG = nG * UW
                    if nG:
                        usbg = usbg_p.tile([SR, wG * D], BF16, tag="usbg")
                        for k in range(nG):
                            o0 = (nD + nV + k) * UW
                            u_ps = ps_u.tile([SR, UW * D], F32, tag="ups")
                            nc.tensor.matmul(
                                u_ps, qT_sb[:, rs],
                                W2til_sb[:, o0 * D : (o0 + UW) * D],
                                start=True, stop=True,
                            )
                            nc.scalar.copy(
                                out=usbg[:, k * UW * D : (k + 1) * UW * D],
                                in_=u_ps,
                            )
                        prodg = prodg_p.tile([SR, wG * D], BF16, tag="prodG")
                        pg = prodg.rearrange("p (o j) -> p o j", o=wG)
                        nc.gpsimd.tensor_mul(
                            pg, usbg.rearrange("p (o j) -> p o j", o=wG),
                            ev[:, oG : oG + wG],
                        )
                        w = D
                        for hi, lo in _tree_levels(D):
                            if w == 2:
                                nc.gpsimd.tensor_add(
                                    node_g[:, oG : oG + wG].unsqueeze(2),
                                    pg[:, :, 0:1], pg[:, :, 1:2],
                                )
                            else:
                                nc.gpsimd.tensor_add(
                                    pg[:, :, 0:hi], pg[:, :, 0:hi],
                                    pg[:, :, lo : lo + hi],
                                )
                            w = lo

                    # keep the neighbor pipeline fed
                    emit_chunk()
                    if st % 2 == 1:
                        emit_chunk()

                while ci < NCH and not skip_nbr:
                    emit_chunk()

                # per-region: transpose-accumulate node terms, close the
                # accumulation group with the bias matmul, then the tail —
                # all emitted per region so region r's tail chain executes as
                # soon as ITS dependencies resolve (PE is in-order: emitting
                # all transposes first would gate region 0's tail on the last
                # sub-tile's DVE tree)
                for r in range(NREG):
                    if not skip_bil:
                        for k in range(SPR):
                            st = r * SPR + k
                            nc.tensor.matmul(
                                aggR[r][:, k * SR : (k + 1) * SR],
                                node_gbs[st],
                                ident_sb[0:SR, 0:SR],
                                start=(skip_nbr and k == 0),
                                stop=False,
                            )
                    nc.tensor.matmul(
                        aggR[r],
                        bbilg_rep[0:1],
                        s63r_sb[:, r * RW : (r + 1) * RW],
                        start=(skip_nbr and skip_bil),
                        stop=True,
                    )
                    e_sb = tailp.tile([D, RW], BF16, tag="e")
                    nc.scalar.activation(out=e_sb, in_=aggR[r], func=AF.Exp,
                                         bias=gb_sb)
                    r_sb = tailp.tile([D, RW], BF16, tag="r")
                    nc.scalar.activation(out=r_sb, in_=aggR[r], func=AF.Relu,
                                         bias=gb_sb)
                    feats = tailp.tile([D, RW], BF16, tag="feats")
                    nc.vector.scalar_tensor_tensor(
                        out=feats, in0=e_sb, scalar=-1.0, in1=r_sb,
                        op0=OP.add, op1=OP.min,
                    )
                    out_ps = ps_misc.tile([1, RW], F32, tag="misc", name="out_ps")
                    nc.tensor.matmul(out_ps, WrT_col, feats, start=True, stop=True)
                    nc.scalar.activation(
                        out=out_row[:, r * RW : (r + 1) * RW], in_=out_ps,
                        func=AF.Identity, bias=br_val,
                    )
                nc.sync.dma_start(out=t_out[:], in_=out_row)

            if repeat == 1:
                emit_body()
            else:
                with tc.For_i(0, repeat, 1):
                    emit_body()

    nc.finalize()
    return nc


def kernel(
    query_emb,
    entity_emb,
    neighbor_embs,
    neighbor_scores,
    Wp,
    bp,
    Wbil,
    bbil,
    Wg,
    g_bias,
    Wr,
    br,
):
    br_val = float(np.asarray(br).reshape(-1)[0])
    if "nc" not in _CACHE:
        _CACHE["nc"] = build_program(br_val)
    nc = _CACHE["nc"]

    bf = ml_dtypes.bfloat16
    q = np.asarray(query_emb, np.float32)
    ent = np.asarray(entity_emb, np.float32)
    nbr = np.asarray(neighbor_embs, np.float32)
    sc = np.asarray(neighbor_scores, np.float32)
    Wg_ = np.asarray(Wg, np.float32)
    Wbil_ = np.asarray(Wbil, np.float32)

    # ---- shared weight prep ----
    # Wtil[p,i,j] = sum_o Wg[p,o] Wbil[o,i,j]; W2til[i, p*D+j] = Wtil[p,i,j]
    Wtil = np.einsum("po,oij->pij", Wg_, Wbil_)
    # j-major columns: U[n, j*D+o] so the DVE tree adds are flat 2D
    W2til_f = Wtil.transpose(1, 2, 0).reshape(D, OJ)
    bbilg = Wg_ @ np.asarray(bbil, np.float32)  # [50]
    WpT_aug = np.zeros((EA, D), np.float32)
    WpT_aug[0:E] = np.asarray(Wp, np.float32).T
    WpT_aug[E] = np.asarray(bp, np.float32)
    # WgK[(db,k), c*D+o] = Wg[o, 2c+db]
    WgT = Wg_.T  # [d, o]
    WgK = np.empty((CR, NCH * D), np.float32)
    for c in range(NCH):
        WgK[:, c * D : (c + 1) * D] = np.repeat(WgT[2 * c : 2 * c + 2], K, axis=0)
    # wpack: WpT0 | WpT1 | bbilg_rep+WrT | WpT2
    wpack = np.zeros((P, 100 + D + 1 + D), np.float32)
    wpack[:, 0:D] = WpT_aug[0:128]
    wpack[:, D : 2 * D] = WpT_aug[128:256]
    wpack[:, 100:150] = bbilg[None, :]
    wpack[0:D, 150] = np.asarray(Wr, np.float32).reshape(-1)
    wpack[0 : EA - 256, 151 : 151 + D] = WpT_aug[256:EA]
    wpack = wpack.astype(bf)
    gb = np.asarray(g_bias, np.float32)

    in_maps = []
    for c in range(N_CORES):
        s = slice(c * NLOC, (c + 1) * NLOC)
        ent_aug = np.zeros((EA, NLOC), np.float32)
        ent_aug[0:E] = ent[s].T
        ent_aug[E] = 1.0
        nbrT = nbr[s].transpose(2, 1, 0).reshape(NCH * CR, NLOC)
        sT = sc[s, 0:K].T  # [63, NLOC]
        f32p = np.zeros((SR, NLOC // SR + 1), np.float32)
        f32p[:, 0 : NLOC // SR] = sc[s, K].reshape(NLOC // SR, SR).T
        f32p[0:D, NLOC // SR] = gb
        s63r = sc[s, K][None, :]  # [1, NLOC]
        in_maps.append(
            {
                "nbrT": np.ascontiguousarray(nbrT).astype(bf),
                "sWgK": np.ascontiguousarray(
                    np.concatenate(
                        [np.concatenate([sT, sT], axis=0), WgK], axis=1
                    )
                ).astype(bf),
                "f32p": np.ascontiguousarray(f32p),
                "s63r": np.ascontiguousarray(s63r).astype(bf),
                "qW": np.ascontiguousarray(
                    np.concatenate([q[s].T, W2til_f], axis=1)
                ).astype(bf),
                "entT0": np.ascontiguousarray(ent_aug[0:128]).astype(bf),
                "entT1": np.ascontiguousarray(ent_aug[128:256]).astype(bf),
                "entT2": np.ascontiguousarray(ent_aug[256:EA]).astype(bf),
                "wpack": wpack,
            }
        )

    _CACHE["last_in_maps"] = in_maps
    res = run_bass_kernel_spmd(nc, in_maps, core_ids=list(range(N_CORES)))
    out = np.concatenate(
        [res.results[c]["out"].reshape(NLOC, 1) for c in range(N_CORES)], axis=0
    )
    return out.astype(np.float32)



# revision 18
# speedup vs baseline: 1.1058x; 1.1058x over previous
"""Trainium2 Bass kernel for NeuralECMModel (gnn_message_passing).

Math (per node n):
  ent  = entity_emb @ Wp.T + bp                                   [N,50]
  node = einsum('ni,oij,nj->no', q, Wbil, ent) + bbil             [N,50]
  wtext= sum_k s[n,k]*nbr[n,k,:] + s[n,63]*node[n,:]              [N,50]
  agg  = wtext @ Wg.T                                             [N,50]
  out  = elu(agg + g_bias) @ Wr.T + br                            [N,1]

Key restructuring (vs naive): Wg is folded into both branches so `agg`
is accumulated directly in PSUM by the PE:
  agg[p,n] = sum_{(d,k)} Wg[p,d]*s[n,k]*nbr[n,k,d]        (PE contraction
             over 25 chunks of the transposed neighbor stream)
           + s63[n]*(q Wtil[p] ent + bbilg[p])            (row-major bilinear,
             transpose-matmul-accumulated into the same PSUM tile)
  with Wtil[p,i,j] = sum_o Wg[p,o]*Wbil[o,i,j], bbilg = Wg @ bbil.

This removes the k-tree reduction from the vector engines entirely; the
score multiply is ONE full-rate bf16 DVE op per 500-node macro tile.

Sharding: pure data parallel over nodes, N=20000 -> 2500 nodes/core x 8.
"""

import numpy as np
import ml_dtypes

import concourse.bass as bass
import concourse.bacc as bacc
import concourse.tile as tile
import concourse.mybir as mybir
from concourse.bass_utils import run_bass_kernel_spmd
from concourse.masks import make_identity

F32 = mybir.dt.float32
BF16 = mybir.dt.bfloat16
OP = mybir.AluOpType
AF = mybir.ActivationFunctionType
AX = mybir.AxisListType

N_CORES = 8
N = 20000
NLOC = N // N_CORES   # 2500
K = 63
D = 50
E = 300
EA = 304              # padded augmented entity rows (300 + ones + 3 zero)
P = 128
SR = 125              # bilinear sub-tile rows
NCH = 25              # neighbor (d,k) chunks of 126 rows
CR = 2 * K            # 126 rows per chunk (2 d's x 63 k's)
OJ = D * D            # 2500

RW = 500              # agg region width (one PSUM bank: 500 f32)
# per-sub-tile U o-chunk assignment (n_direct_psum_dve, n_sbuf_dve, n_pool)
UPLAN = ((0, 5, 0),)   # all U chunks: ACT egress copy -> DVE mul+reduce
# snbr chunk engine: True -> Pool, False -> DVE
SNBR_POOL = tuple(False for _ in range(25))
# U psum chunk width in o's (each *D wide); one PSUM bank per chunk so every
# matmul output is bank-aligned
UW = 10

_CACHE = {}


def _tree_levels(w):
    """Pairwise-halving splits: [(hi, lo), ...] meaning x[0:hi] += x[lo:lo+hi]."""
    out = []
    while w > 1:
        lo = (w + 1) // 2
        hi = w - lo
        out.append((hi, lo))
        w = lo
    return out


def build_program(br_val: float, skip_bil=False, skip_nbr=False, dump_ng=False,
                  repeat=1, uplan=None, dve_tree=False, snbr_pool=None):
    """repeat>1 wraps the whole per-run body (including all input streaming)
    in a hardware loop - used by test.py to time steady-state per-execution
    HW time with the ~3.6ms axon launch round-trip amortized away.

    v2 structure (single pass over all NLOC nodes):
      - neighbor stream DMA'd chunk-major: 25 chunks of [126, 2500] with 5KB
        contiguous rows (vs 0.5-1KB rows of the macro-tile slicing).
      - agg accumulated in 5 persistent PSUM bank regions [50, 500] covering
        all 2500 nodes; ONE tail (elu + Wr head) at the end.
      - bilinear j-reduction on DVE via a single tensor_reduce(X) instead of
        a 6-instruction pairwise tree; Pool keeps the tree for its o-share.
      - per-sub-tile engine assignment of the 5 U o-chunks: chunk 0 is
        consumed by DVE straight from PSUM (saves the ACT egress copy),
        the rest split DVE/Pool via an alternating pattern.
    """
    nc = bacc.Bacc("TRN2", debug=False, num_devices=N_CORES)

    # ---- per-core inputs (layouts unchanged from v1) ----
    t_nbrT = nc.dram_tensor("nbrT", [NCH * CR, NLOC], BF16, kind="ExternalInput")
    t_f32p = nc.dram_tensor("f32p", [SR, NLOC // SR + 1], F32, kind="ExternalInput")
    t_qW = nc.dram_tensor("qW", [D, NLOC + OJ], BF16, kind="ExternalInput")
    t_entT0 = nc.dram_tensor("entT0", [128, NLOC], BF16, kind="ExternalInput")
    t_entT1 = nc.dram_tensor("entT1", [128, NLOC], BF16, kind="ExternalInput")
    t_entT2 = nc.dram_tensor("entT2", [EA - 256, NLOC], BF16, kind="ExternalInput")
    t_wpack = nc.dram_tensor("wpack", [128, 100 + D + 1 + D], BF16, kind="ExternalInput")
    t_sWgK = nc.dram_tensor("sWgK", [CR, NLOC + NCH * D], BF16, kind="ExternalInput")
    t_s63r = nc.dram_tensor("s63r", [1, NLOC], BF16, kind="ExternalInput")
    t_out = nc.dram_tensor("out", [1, NLOC], F32, kind="ExternalOutput")

    NREG = NLOC // RW          # 5 agg regions of 500 nodes (1 PSUM bank each)
    NSUB = NLOC // SR          # 20 bilinear sub-tiles
    SPR = RW // SR             # 4 sub-tiles per region

    with tile.TileContext(nc) as tc:
        with (
            tc.tile_pool(name="res", bufs=1) as res,
            tc.tile_pool(name="nbrp", bufs=6) as nbrp,
            tc.tile_pool(name="snbrp", bufs=4) as snbrp,
            tc.tile_pool(name="usbd", bufs=3) as usbd_p,
            tc.tile_pool(name="usbg", bufs=3) as usbg_p,
            tc.tile_pool(name="prodd", bufs=3) as prodd_p,
            tc.tile_pool(name="prodg", bufs=3) as prodg_p,
            tc.tile_pool(name="small", bufs=8) as small,
            tc.tile_pool(name="nodegs", bufs=20) as nodegs,
            tc.tile_pool(name="tailp", bufs=3) as tailp,
            tc.tile_pool(name="ps_agg", bufs=1, space="PSUM") as ps_agg,
            tc.tile_pool(name="ps_u", bufs=2, space="PSUM") as ps_u,
            tc.tile_pool(name="ps_misc", bufs=1, space="PSUM") as ps_misc,
            nc.allow_low_precision(reason="bf16 bilinear reduce; node term is ~10% of signal"),
        ):
            ident_sb = res.tile([P, P], BF16)
            make_identity(nc, ident_sb)

            def emit_body():
                # ---- residents: sWgK first (feeds snbr_0, DVE's first op) ----
                sWgK_sb = res.tile([CR, NLOC + NCH * D], BF16)
                nc.sync.dma_start(out=sWgK_sb, in_=t_sWgK[:])
                qW_sb = res.tile([D, NLOC + OJ], BF16)
                nc.sync.dma_start(out=qW_sb, in_=t_qW[:])
                wpack_sb = res.tile([P, 100 + D + 1 + D], BF16)
                nc.sync.dma_start(out=wpack_sb, in_=t_wpack[:])
                f32p_sb = res.tile([SR, NLOC // SR + 1], F32)
                nc.sync.dma_start(out=f32p_sb, in_=t_f32p[:])
                entT_sb = [
                    res.tile([128, NLOC], BF16, tag="entT0", name="entT0_sb"),
                    res.tile([128, NLOC], BF16, tag="entT1", name="entT1_sb"),
                    res.tile([EA - 256, NLOC], BF16, tag="entT2", name="entT2_sb"),
                ]
                for sb, t in zip(entT_sb, (t_entT0, t_entT1, t_entT2)):
                    nc.sync.dma_start(out=sb, in_=t[:])
                s63r_sb = res.tile([1, NLOC], BF16)
                nc.sync.dma_start(out=s63r_sb, in_=t_s63r[:])
                out_row = res.tile([1, NLOC], F32)

                sTrep_sb = sWgK_sb[:, 0:NLOC]
                WgK_sb = sWgK_sb[:, NLOC : NLOC + NCH * D]
                qT_sb = qW_sb[:, 0:NLOC]
                W2til_sb = qW_sb[:, NLOC : NLOC + OJ]
                WpT_sb = [
                    wpack_sb[:, 0:D],
                    wpack_sb[:, D : 2 * D],
                    wpack_sb[0 : EA - 256, 151 : 151 + D],
                ]
                bbilg_rep = wpack_sb[:, 100:150]
                WrT_col = wpack_sb[0:D, 150:151]
                s63p_sb = f32p_sb[:, 0 : NLOC // SR]
                gb_sb = f32p_sb[0:D, NLOC // SR : NLOC // SR + 1]

                # ---- neighbor chunk DMAs (self-throttled by pool bufs) ----
                nbr_c = []
                for c in range(NCH if not skip_nbr else 0):
                    t = nbrp.tile([CR, NLOC], BF16, tag="nbr", name=f"nbr{c}")
                    nc.sync.dma_start(out=t, in_=t_nbrT[c * CR : (c + 1) * CR, :])
                    nbr_c.append(t)

                # persistent agg regions
                aggR = [
                    ps_agg.tile([D, RW], F32, tag=f"agg{r}", name=f"agg{r}")
                    for r in range(NREG)
                ]

                def emit_snbr(c):
                    st_ = snbrp.tile([CR, NLOC], BF16, tag="snbr", name=f"snbr{c}")
                    sp = snbr_pool if snbr_pool is not None else SNBR_POOL
                    eng = nc.gpsimd if sp[c] else nc.vector
                    eng.tensor_mul(st_, nbr_c[c], sTrep_sb)
                    return st_

                def emit_nbr_mm(c, snbr_t):
                    for r in range(NREG):
                        nc.tensor.matmul(
                            aggR[r],
                            WgK_sb[:, c * D : (c + 1) * D],
                            snbr_t[:, r * RW : (r + 1) * RW],
                            start=(c == 0),
                            stop=False,
                        )

                # ---- interleaved bilinear sub-tiles + neighbor chunks ----
                ci = 0  # next neighbor chunk to emit
                node_gbs = []

                def emit_chunk():
                    nonlocal ci
                    if not skip_nbr and ci < NCH:
                        emit_nbr_mm(ci, emit_snbr(ci))
                        ci += 1

                for st in range(NSUB if not skip_bil else 0):
                    r0 = st * SR
                    rs = slice(r0, r0 + SR)

                    ent_ps = ps_misc.tile([SR, D], F32, tag="misc", name="ent_ps")
                    for c in range(3):
                        nc.tensor.matmul(
                            ent_ps,
                            entT_sb[c][:, rs],
                            WpT_sb[c],
                            start=(c == 0),
                            stop=(c == 2),
                        )
                    ents = small.tile([SR, D], BF16, tag="ents")
                    nc.scalar.activation(
                        out=ents, in_=ent_ps, func=AF.Copy,
                        scale=s63p_sb[:, st : st + 1],
                    )
                    ev = ents.unsqueeze(1).broadcast_to([SR, D, D])

                    # U chunk plan for this sub-tile: (n_direct, n_dve, n_pool)
                    _up = uplan if uplan is not None else UPLAN
                    nD, nV, nG = _up[st % len(_up)]
                    node_g = nodegs.tile([SR, D], BF16, tag="node_g",
                                         name=f"node_g{st}")
                    node_gbs.append(node_g)

                    # direct-PSUM chunks: DVE consumes u_ps f32 (no ACT copy)
                    for k in range(nD):
                        o0 = k * UW
                        u_ps = ps_u.tile([SR, UW * D], F32, tag="ups")
                        nc.tensor.matmul(
                            u_ps, qT_sb[:, rs],
                            W2til_sb[:, o0 * D : (o0 + UW) * D],
                            start=True, stop=True,
                        )
                        prodd = prodd_p.tile([SR, UW * D], BF16, tag="prodD")
                        pv = prodd.rearrange("p (o j) -> p o j", o=UW)
                        nc.vector.tensor_mul(
                            pv, u_ps.rearrange("p (o j) -> p o j", o=UW),
                            ev[:, o0 : o0 + UW],
                        )
                        nc.vector.tensor_reduce(
                            node_g[:, o0 : o0 + UW].unsqueeze(2), pv,
                            axis=AX.X, op=OP.add,
                        )

                    # DVE-via-SBUF chunks (ACT egress copy, batched mul+reduce)
                    oV = nD * UW
                    wV = nV * UW
                    if nV:
                        usbd = usbd_p.tile([SR, wV * D], BF16, tag="usbd")
                        for k in range(nV):
                            o0 = (nD + k) * UW
                            u_ps = ps_u.tile([SR, UW * D], F32, tag="ups")
                            nc.tensor.matmul(
                                u_ps, qT_sb[:, rs],
                                W2til_sb[:, o0 * D : (o0 + UW) * D],
                                start=True, stop=True,
                            )
                            nc.scalar.copy(
                                out=usbd[:, k * UW * D : (k + 1) * UW * D],
                                in_=u_ps,
                            )
                        prodv = prodd_p.tile([SR, wV * D], BF16, tag="prodV")
                        pv = prodv.rearrange("p (o j) -> p o j", o=wV)
                        nc.vector.tensor_mul(
                            pv, usbd.rearrange("p (o j) -> p o j", o=wV),
                            ev[:, oV : oV + wV],
                        )
                        if dve_tree:
                            w = D
                            for hi, lo in _tree_levels(D):
                                if w == 2:
                                    nc.vector.tensor_add(
                                        node_g[:, oV : oV + wV].unsqueeze(2),
                                        pv[:, :, 0:1], pv[:, :, 1:2],
                                    )
                                else:
                                    nc.vector.tensor_add(
                                        pv[:, :, 0:hi], pv[:, :, 0:hi],
                                        pv[:, :, lo : lo + hi],
                                    )
                                w = lo
                        else:
                            nc.vector.tensor_reduce(
                                node_g[:, oV : oV + wV].unsqueeze(2), pv,
                                axis=AX.X, op=OP.add,
                            )

                    # Pool chunks (ACT egress copy, mul + pairwise tree)
                    oG = (nD + nV) * UW
                    wG = nG * UW
                    if nG:
                        usbg = usbg_p.tile([SR, wG * D], BF16, tag="usbg")
                        for k in range(nG):
                            o0 = (nD + nV + k) * UW
                            u_ps = ps_u.tile([SR, UW * D], F32, tag="ups")
                            nc.tensor.matmul(
                                u_ps, qT_sb[:, rs],
                                W2til_sb[:, o0 * D : (o0 + UW) * D],
                                start=True, stop=True,
                            )
                            nc.scalar.copy(
                                out=usbg[:, k * UW * D : (k + 1) * UW * D],
                                in_=u_ps,
                            )
                        prodg = prodg_p.tile([SR, wG * D], BF16, tag="prodG")
                        pg = prodg.rearrange("p (o j) -> p o j", o=wG)
                        nc.gpsimd.tensor_mul(
                            pg, usbg.rearrange("p (o j) -> p o j", o=wG),
                            ev[:, oG : oG + wG],
                        )
                        w = D
                        for hi, lo in _tree_levels(D):
                            if w == 2:
                                nc.gpsimd.tensor_add(
                                    node_g[:, oG : oG + wG].unsqueeze(2),
                                    pg[:, :, 0:1], pg[:, :, 1:2],
                                )
                            else:
                                nc.gpsimd.tensor_add(
                                    pg[:, :, 0:hi], pg[:, :, 0:hi],
                                    pg[:, :, lo : lo + hi],
                                )
                            w = lo

                    # keep the neighbor pipeline fed
                    emit_chunk()
                    if st % 2 == 1:
                        emit_chunk()

                while ci < NCH and not skip_nbr:
                    emit_chunk()

                # per-region: transpose-accumulate node terms, close the
                # accumulation group with the bias matmul, then the tail —
                # all emitted per region so region r's tail chain executes as
                # soon as ITS dependencies resolve (PE is in-order: emitting
                # all transposes first would gate region 0's tail on the last
                # sub-tile's DVE tree)
                for r in range(NREG):
                    if not skip_bil:
                        for k in range(SPR):
                            st = r * SPR + k
                            nc.tensor.matmul(
                                aggR[r][:, k * SR : (k + 1) * SR],
                                node_gbs[st],
                                ident_sb[0:SR, 0:SR],
                                start=(skip_nbr and k == 0),
                                stop=False,
                            )
                    nc.tensor.matmul(
                        aggR[r],
                        bbilg_rep[0:1],
                        s63r_sb[:, r * RW : (r + 1) * RW],
                        start=(skip_nbr and skip_bil),
                        stop=True,
                    )
                    e_sb = tailp.tile([D, RW], BF16, tag="e")
                    nc.scalar.activation(out=e_sb, in_=aggR[r], func=AF.Exp,
                                         bias=gb_sb)
                    r_sb = tailp.tile([D, RW], BF16, tag="r")
                    nc.scalar.activation(out=r_sb, in_=aggR[r], func=AF.Relu,
                                         bias=gb_sb)
                    feats = tailp.tile([D, RW], BF16, tag="feats")
                    nc.vector.scalar_tensor_tensor(
                        out=feats, in0=e_sb, scalar=-1.0, in1=r_sb,
                        op0=OP.add, op1=OP.min,
                    )
                    out_ps = ps_misc.tile([1, RW], F32, tag="misc", name="out_ps")
                    nc.tensor.matmul(out_ps, WrT_col, feats, start=True, stop=True)
                    nc.scalar.activation(
                        out=out_row[:, r * RW : (r + 1) * RW], in_=out_ps,
                        func=AF.Identity, bias=br_val,
                    )
                nc.sync.dma_start(out=t_out[:], in_=out_row)

            if repeat == 1:
                emit_body()
            else:
                with tc.For_i(0, repeat, 1):
                    emit_body()

    nc.finalize()
    return nc


def kernel(
    query_emb,
    entity_emb,
    neighbor_embs,
    neighbor_scores,
    Wp,
    bp,
    Wbil,
    bbil,
    Wg,
    g_bias,
    Wr,
    br,
):
    br_val = float(np.asarray(br).reshape(-1)[0])
    if "nc" not in _CACHE:
        _CACHE["nc"] = build_program(br_val)
    nc = _CACHE["nc"]

    bf = ml_dtypes.bfloat16
    q = np.asarray(query_emb, np.float32)
    ent = np.asarray(entity_emb, np.float32)
    nbr = np.asarray(neighbor_embs, np.float32)
    sc = np.asarray(neighbor_scores, np.float32)
    Wg_ = np.asarray(Wg, np.float32)
    Wbil_ = np.asarray(Wbil, np.float32)

    # ---- shared weight prep ----
    # Wtil[p,i,j] = sum_o Wg[p,o] Wbil[o,i,j]; W2til[i, p*D+j] = Wtil[p,i,j]
    Wtil = np.einsum("po,oij->pij", Wg_, Wbil_)
    W2til_f = Wtil.transpose(1, 0, 2).reshape(D, OJ)
    bbilg = Wg_ @ np.asarray(bbil, np.float32)  # [50]
    WpT_aug = np.zeros((EA, D), np.float32)
    WpT_aug[0:E] = np.asarray(Wp, np.float32).T
    WpT_aug[E] = np.asarray(bp, np.float32)
    # WgK[(db,k), c*D+o] = Wg[o, 2c+db]
    WgT = Wg_.T  # [d, o]
    WgK = np.empty((CR, NCH * D), np.float32)
    for c in range(NCH):
        WgK[:, c * D : (c + 1) * D] = np.repeat(WgT[2 * c : 2 * c + 2], K, axis=0)
    # wpack: WpT0 | WpT1 | bbilg_rep+WrT | WpT2
    wpack = np.zeros((P, 100 + D + 1 + D), np.float32)
    wpack[:, 0:D] = WpT_aug[0:128]
    wpack[:, D : 2 * D] = WpT_aug[128:256]
    wpack[:, 100:150] = bbilg[None, :]
    wpack[0:D, 150] = np.asarray(Wr, np.float32).reshape(-1)
    wpack[0 : EA - 256, 151 : 151 + D] = WpT_aug[256:EA]
    wpack = wpack.astype(bf)
    gb = np.asarray(g_bias, np.float32)

    in_maps = []
    for c in range(N_CORES):
        s = slice(c * NLOC, (c + 1) * NLOC)
        ent_aug = np.zeros((EA, NLOC), np.float32)
        ent_aug[0:E] = ent[s].T
        ent_aug[E] = 1.0
        nbrT = nbr[s].transpose(2, 1, 0).reshape(NCH * CR, NLOC)
        sT = sc[s, 0:K].T  # [63, NLOC]
        f32p = np.zeros((SR, NLOC // SR + 1), np.float32)
        f32p[:, 0 : NLOC // SR] = sc[s, K].reshape(NLOC // SR, SR).T
        f32p[0:D, NLOC // SR] = gb
        s63r = sc[s, K][None, :]  # [1, NLOC]
        in_maps.append(
            {
                "nbrT": np.ascontiguousarray(nbrT).astype(bf),
                "sWgK": np.ascontiguousarray(
                    np.concatenate(
                        [np.concatenate([sT, sT], axis=0), WgK], axis=1
                    )
                ).astype(bf),
                "f32p": np.ascontiguousarray(f32p),
                "s63r": np.ascontiguousarray(s63r).astype(bf),
                "qW": np.ascontiguousarray(
                    np.concatenate([q[s].T, W2til_f], axis=1)
                ).astype(bf),
                "entT0": np.ascontiguousarray(ent_aug[0:128]).astype(bf),
                "entT1": np.ascontiguousarray(ent_aug[128:256]).astype(bf),
                "entT2": np.ascontiguousarray(ent_aug[256:EA]).astype(bf),
                "wpack": wpack,
            }
        )

    _CACHE["last_in_maps"] = in_maps
    res = run_bass_kernel_spmd(nc, in_maps, core_ids=list(range(N_CORES)))
    out = np.concatenate(
        [res.results[c]["out"].reshape(NLOC, 1) for c in range(N_CORES)], axis=0
    )
    return out.astype(np.float32)

